# revision 14
# baseline (speedup 1.0000x reference)
"""Multi-head attention (b=4, l=2048, d=1024, h=16) on 8 TRN2 NeuronCores.

Sharding: batch (4-way) x query-sequence (2-way) => 8 shards, no collectives.
Each core computes, for its (batch, query-half):
  - K/V projections for the full 2048-key sequence (duplicated across the
    2 cores sharing a batch), Q projection for its 1024 queries.
  - Scores in transposed orientation sT[k, q] = (k_h q_h^T) so the PV matmul
    contracts k on partitions; softmax without max-subtraction (scores ~N(0,1));
    the exp row-sum rides the PV matmul as a ones-column of v (M=65).
  - Output projection, writing its own [1024, 1024] slice of the output.

Compute dtype is fp16 (inputs cast on-device during DMA): matmuls stream at
1 cycle/row with fp32 PSUM accumulation, activations get hardware DMA
transpose (2-byte dtype), and head-paired matmuls run concurrently in the
64-row-tiled PE array. Softmax normalization stays in fp32.

Weights are passed to the device pre-transposed ([d_in, d_out]) - a host-side
layout choice.
"""

import sys
import types

import numpy as np

B, L, D, H, DK = 4, 2048, 1024, 16, 64
LQ = L // 2          # queries per core
P = 128              # partitions
DCH = D // P         # 8 d_in chunks
NPAIR = H // 2       # 8 head pairs
NKT = L // P         # 16 key tiles
N_CORES = 8
SCALE = 1.0 / np.sqrt(DK)

_NC_CACHE = {}
DEBUG_OUTPUTS = False
USE_TILE_POS = True


def _ensure_axon_hooks():
    """Register the NTFF profile hook module if the image's antenv lacks it."""
    try:
        import antenv  # noqa: F401
        from antenv import axon_hooks  # noqa: F401
        return
    except ImportError:
        pass
    try:
        import antenv

        mod = types.ModuleType("antenv.axon_hooks")
        mod._HOOK = None

        def set_axon_ntff_profile_hook(h):
            mod._HOOK = h

        def get_axon_ntff_profile_hook():
            return mod._HOOK

        mod.set_axon_ntff_profile_hook = set_axon_ntff_profile_hook
        mod.get_axon_ntff_profile_hook = get_axon_ntff_profile_hook
        sys.modules["antenv.axon_hooks"] = mod
        antenv.axon_hooks = mod
        from trn_agent_boot.trn_boot import _ntff_profile_via_ctypes

        set_axon_ntff_profile_hook(
            _ntff_profile_via_ctypes("/opt/axon/libaxon_pjrt.so")
        )
    except Exception:
        pass


def build_nc():
    import concourse.tile as tile
    from concourse import bacc, mybir
    from contextlib import ExitStack

    f32 = mybir.dt.float32
    f16 = mybir.dt.float16
    Exp = mybir.ActivationFunctionType.Exp

    nc = bacc.Bacc(
        "TRN2",
        target_bir_lowering=False,
        debug=False,
        enable_asserts=False,
        num_devices=N_CORES,
    )

    Qc = nc.dram_tensor("Qc", [LQ, D], f32, kind="ExternalInput").ap()
    Kc = nc.dram_tensor("Kc", [L, D], f32, kind="ExternalInput").ap()
    Vc = nc.dram_tensor("Vc", [L, D], f32, kind="ExternalInput").ap()
    WQT = nc.dram_tensor("WQT", [D, D], f32, kind="ExternalInput").ap()
    WKT = nc.dram_tensor("WKT", [D, D], f32, kind="ExternalInput").ap()
    WVT = nc.dram_tensor("WVT", [D, D], f32, kind="ExternalInput").ap()
    WOT = nc.dram_tensor("WOT", [D, D], f32, kind="ExternalInput").ap()
    Yc = nc.dram_tensor("Yc", [LQ, D], f32, kind="ExternalOutput").ap()
    dbg = {}
    if DEBUG_OUTPUTS:
        dbg["qT"] = nc.dram_tensor("dbg_qT", [D, LQ], f16, kind="ExternalOutput").ap()
        dbg["kT"] = nc.dram_tensor("dbg_kT", [D, L], f16, kind="ExternalOutput").ap()
        dbg["v"] = nc.dram_tensor("dbg_v", [L, NPAIR * 130], f16, kind="ExternalOutput").ap()
        dbg["oT"] = nc.dram_tensor("dbg_oT", [D, LQ], f16, kind="ExternalOutput").ap()

    with tile.TileContext(nc) as tc, ExitStack() as top:
        dram = top.enter_context(tc.tile_pool(name="dram", bufs=1, space="DRAM"))
        Qh = dram.tile([LQ, D], f16)
        Kh = dram.tile([L, D], f16)
        Vh = dram.tile([L, D], f16)
        qT_d = dram.tile([D, LQ], f16)          # q^T: [d_out, lq]
        kT_d = dram.tile([D, L], f16)           # k^T: [d_out, lk]
        v_d = dram.tile([L, NPAIR * 130], f16)  # v: [lk, pair-blocks A|1|B|1]

        # cast inputs to fp16 in DRAM (SWDGE casts during the copy)
        for half in range(2):
            r0, r1 = half * (L // 2), (half + 1) * (L // 2)
            nc.gpsimd.dma_start(Vh[r0:r1, :], Vc[r0:r1, :])
            nc.gpsimd.dma_start(Kh[r0:r1, :], Kc[r0:r1, :])
        nc.gpsimd.dma_start(Qh[:, :], Qc[:, :])

        wpool = top.enter_context(tc.tile_pool(name="w", bufs=2))

        def load_weight(WT):
            w = wpool.tile([P, DCH * D], f16)
            nc.gpsimd.dma_start(
                w[:].rearrange("p (c o) -> p c o", c=DCH),
                WT.rearrange("(c p) o -> p c o", p=P),
            )
            return w

        # ---------------- Phase A: transposed loads + projections -----------
        with ExitStack() as pa:
            xtp = pa.enter_context(tc.tile_pool(name="xt", bufs=2))
            stg = pa.enter_context(tc.tile_pool(name="stg", bufs=4))
            ppsum = pa.enter_context(
                tc.tile_pool(name="ppsum", bufs=4, space="PSUM")
            )

            def load_xt(Xh, l0, lblk):
                """xt[p, c*lblk + j] = Xh[l0+j, c*P+p] via hardware DMA transpose."""
                xt = xtp.tile([P, DCH * lblk], f16)
                xt3 = xt[:].rearrange("p (c l) -> p c l", c=DCH)
                for c in range(DCH):
                    nc.sync.dma_start(
                        xt3[:, c, :],
                        Xh[l0 : l0 + lblk, P * c : P * (c + 1)],
                        transpose=True,
                    )
                return xt

            # V phase: natural-layout v with interleaved fp16 ones columns
            wv = load_weight(WVT)
            for half in range(2):
                l0 = half * (L // 2)
                xt = load_xt(Vh, l0, L // 2)
                xt3 = xt[:].rearrange("p (c l) -> p c l", c=DCH)
                for i in range(8):
                    for nb in range(2):
                        pp = ppsum.tile([P, 512], f32)
                        for c in range(DCH):
                            nc.tensor.matmul(
                                pp[:],
                                xt3[:, c, P * i : P * (i + 1)],
                                wv[:, c * D + 512 * nb : c * D + 512 * (nb + 1)],
                                start=(c == 0),
                                stop=(c == DCH - 1),
                            )
                        vs = stg.tile([P, 520], f16)
                        vs4 = vs[:].rearrange("p (a h s) -> p a h s", h=2, s=65)
                        nc.vector.tensor_copy(
                            vs4[:, :, :, 0:64],
                            pp[:].rearrange("p (a h s) -> p a h s", h=2, s=64),
                        )
                        nc.gpsimd.memset(vs4[:, :, :, 64:65], 1.0)
                        row = l0 + P * i
                        nc.sync.dma_start(
                            v_d[row : row + P, 520 * nb : 520 * (nb + 1)], vs[:]
                        )

            # K phase: kT[d_out, l]
            wk = load_weight(WKT)
            for half in range(2):
                l0 = half * (L // 2)
                xt = load_xt(Kh, l0, L // 2)
                xt3 = xt[:].rearrange("p (c l) -> p c l", c=DCH)
                for pr in range(NPAIR):
                    for lb in range(2):
                        pp = ppsum.tile([P, 512], f32)
                        for c in range(DCH):
                            nc.tensor.matmul(
                                pp[:],
                                wk[:, c * D + P * pr : c * D + P * (pr + 1)],
                                xt3[:, c, 512 * lb : 512 * (lb + 1)],
                                start=(c == 0),
                                stop=(c == DCH - 1),
                            )
                        ks = stg.tile([P, 512], f16, tag="ks")
                        nc.vector.tensor_copy(ks[:], pp[:])
                        nc.sync.dma_start(
                            kT_d[
                                P * pr : P * (pr + 1),
                                l0 + 512 * lb : l0 + 512 * (lb + 1),
                            ],
                            ks[:],
                        )

            # Q phase: qT[d_out, lq]
            wq = load_weight(WQT)
            xt = load_xt(Qh, 0, LQ)
            xt3 = xt[:].rearrange("p (c l) -> p c l", c=DCH)
            for pr in range(NPAIR):
                for lb in range(2):
                    pp = ppsum.tile([P, 512], f32)
                    for c in range(DCH):
                        nc.tensor.matmul(
                            pp[:],
                            wq[:, c * D + P * pr : c * D + P * (pr + 1)],
                            xt3[:, c, 512 * lb : 512 * (lb + 1)],
                            start=(c == 0),
                            stop=(c == DCH - 1),
                        )
                    qs_t = stg.tile([P, 512], f16, tag="ks")
                    nc.vector.tensor_copy(qs_t[:], pp[:])
                    nc.sync.dma_start(
                        qT_d[P * pr : P * (pr + 1), 512 * lb : 512 * (lb + 1)],
                        qs_t[:],
                    )

        if DEBUG_OUTPUTS:
            nc.gpsimd.dma_start(dbg["qT"], qT_d[:])
            nc.gpsimd.dma_start(dbg["kT"], kT_d[:])
            nc.gpsimd.dma_start(dbg["v"], v_d[:])

        # ---------------- Phase B: attention ----------------
        with ExitStack() as pb:
            katt = pb.enter_context(tc.tile_pool(name="katt", bufs=2))
            qatt = pb.enter_context(tc.tile_pool(name="qatt", bufs=2))
            vatt = pb.enter_context(tc.tile_pool(name="vatt", bufs=2))
            epool = pb.enter_context(tc.tile_pool(name="epool", bufs=3))
            npool = pb.enter_context(tc.tile_pool(name="npool", bufs=4))
            rpool = pb.enter_context(tc.tile_pool(name="rpool", bufs=4))
            otp = pb.enter_context(tc.tile_pool(name="otp", bufs=NPAIR))
            spsum = pb.enter_context(
                tc.tile_pool(name="spsum", bufs=2, space="PSUM")
            )
            pvpsum = pb.enter_context(
                tc.tile_pool(name="pvpsum", bufs=4, space="PSUM")
            )

            npv = 4 if USE_TILE_POS else 2
            ot_tiles = []
            for pr in range(NPAIR):
                kt = katt.tile([P, L], f16)
                nc.sync.dma_start(kt[:], kT_d[P * pr : P * (pr + 1), :])
                qt = qatt.tile([P, LQ], f16)
                nc.sync.dma_start(qt[:], qT_d[P * pr : P * (pr + 1), :])
                v1 = vatt.tile([P, NKT * 130], f16)
                nc.sync.dma_start(
                    v1[:].rearrange("p (t c) -> p t c", c=130),
                    v_d[:].rearrange("(t p) (a c) -> p t a c", p=P, c=130)[
                        :, :, pr, :
                    ],
                )
                ot = otp.tile([P, LQ], f16)
                ot_tiles.append(ot)

                for qs in range(2):
                    qsl = qt[:, 512 * qs : 512 * (qs + 1)]
                    # tile_pos: A-lo, A-hi, B-lo, B-hi; else: A, B
                    pvs = [
                        pvpsum.tile([P, 512], f32, tag="pv", name=f"pv{pr}_{qs}_{i}")
                        for i in range(npv)
                    ]
                    for g in range(NKT):
                        sab = spsum.tile([P, 1024], f32)
                        st, sp = (g == 0), (g == NKT - 1)
                        for h in range(2):
                            ksl = kt[64 * h : 64 * (h + 1), P * g : P * (g + 1)]
                            qslh = qsl[64 * h : 64 * (h + 1), :]
                            if USE_TILE_POS:
                                nc.tensor.matmul(
                                    sab[:, 512 * h : 512 * (h + 1)],
                                    ksl, qslh, start=True, stop=True,
                                    tile_position=(64 * h, 0),
                                )
                            else:
                                nc.tensor.matmul(
                                    sab[:, 512 * h : 512 * (h + 1)],
                                    ksl, qslh, start=True, stop=True,
                                )
                        e = epool.tile([P, 1024], f16)
                        nc.scalar.activation(
                            e[:], sab[:], Exp, scale=float(SCALE)
                        )
                        for h in range(2):
                            vsl = v1[:, 130 * g + 65 * h : 130 * g + 65 * (h + 1)]
                            esl = e[:, 512 * h : 512 * (h + 1)]
                            if USE_TILE_POS:
                                nc.tensor.matmul(
                                    pvs[2 * h][0:65, :],
                                    vsl[0:64, :], esl[0:64, :],
                                    start=st, stop=sp, tile_position=(0, 0),
                                )
                                nc.tensor.matmul(
                                    pvs[2 * h + 1][0:65, :],
                                    vsl[64:128, :], esl[64:128, :],
                                    start=st, stop=sp, tile_position=(64, 0),
                                )
                            else:
                                nc.tensor.matmul(
                                    pvs[h][0:65, :], vsl, esl,
                                    start=st, stop=sp,
                                )
                    # normalize: rows 0:64 = sum(exp*v), row 64 = sum(exp)
                    for h in range(2):
                        tmp = npool.tile([P, 512], f32)
                        if USE_TILE_POS:
                            nc.vector.tensor_copy(tmp[0:65, :], pvs[2 * h][0:65, :])
                            nc.vector.tensor_add(
                                tmp[0:65, :], tmp[0:65, :], pvs[2 * h + 1][0:65, :]
                            )
                        else:
                            nc.vector.tensor_copy(tmp[0:65, :], pvs[h][0:65, :])
                        # custom DVE ops + partition_broadcast need base 0
                        srow = rpool.tile([1, 512], f32)
                        nc.vector.tensor_copy(srow[0:1, :], tmp[64:65, :])
                        rec = rpool.tile([1, 512], f32)
                        nc.vector.reciprocal_approx_fast(rec[0:1, :], srow[0:1, :])
                        bc = rpool.tile([64, 512], f32)
                        nc.gpsimd.partition_broadcast(bc[:, :], rec[0:1, :])
                        nc.vector.tensor_mul(
                            ot[64 * h : 64 * (h + 1), 512 * qs : 512 * (qs + 1)],
                            tmp[0:64, :],
                            bc[:, :],
                        )

            if DEBUG_OUTPUTS:
                for pr in range(NPAIR):
                    nc.gpsimd.dma_start(
                        dbg["oT"][P * pr : P * (pr + 1), :], ot_tiles[pr][:]
                    )

            # ---------------- Phase C: output projection ----------------
            wo = load_weight(WOT)
            ystg = pb.enter_context(tc.tile_pool(name="ystg", bufs=3))
            for qt_i in range(LQ // P):
                for nb in range(2):
                    pp = spsum.tile([P, 512], f32, tag="sab")
                    for pr in range(NPAIR):
                        nc.tensor.matmul(
                            pp[:],
                            ot_tiles[pr][:, P * qt_i : P * (qt_i + 1)],
                            wo[:, pr * D + 512 * nb : pr * D + 512 * (nb + 1)],
                            start=(pr == 0),
                            stop=(pr == NPAIR - 1),
                        )
                    ys = ystg.tile([P, 512], f32)
                    nc.vector.tensor_copy(ys[:], pp[:])
                    nc.sync.dma_start(
                        Yc[P * qt_i : P * (qt_i + 1), 512 * nb : 512 * (nb + 1)],
                        ys[:],
                    )

    nc.compile()
    return nc


def get_nc():
    if "nc" not in _NC_CACHE:
        _NC_CACHE["nc"] = build_nc()
    return _NC_CACHE["nc"]


def make_in_maps(Q, K, V, WQ, WK, WV, WO):
    Q = np.asarray(Q, dtype=np.float32)
    K = np.asarray(K, dtype=np.float32)
    V = np.asarray(V, dtype=np.float32)
    WQT = np.ascontiguousarray(np.asarray(WQ, dtype=np.float32).T)
    WKT = np.ascontiguousarray(np.asarray(WK, dtype=np.float32).T)
    WVT = np.ascontiguousarray(np.asarray(WV, dtype=np.float32).T)
    WOT = np.ascontiguousarray(np.asarray(WO, dtype=np.float32).T)
    in_maps = []
    for c in range(N_CORES):
        b, half = c // 2, c % 2
        in_maps.append(
            {
                "Qc": np.ascontiguousarray(Q[b, half * LQ : (half + 1) * LQ, :]),
                "Kc": np.ascontiguousarray(K[b]),
                "Vc": np.ascontiguousarray(V[b]),
                "WQT": WQT,
                "WKT": WKT,
                "WVT": WVT,
                "WOT": WOT,
            }
        )
    return in_maps


def kernel(Q, K, V, WQ, WK, WV, WO, trace=False):
    _ensure_axon_hooks()
    from concourse.bass_utils import run_bass_kernel_spmd

    nc = get_nc()
    in_maps = make_in_maps(Q, K, V, WQ, WK, WV, WO)
    res = run_bass_kernel_spmd(
        nc, in_maps, core_ids=list(range(N_CORES)), trace=trace
    )
    out = np.empty((B, L, D), dtype=np.float32)
    for c in range(N_CORES):
        b, half = c // 2, c % 2
        out[b, half * LQ : (half + 1) * LQ, :] = res.results[c]["Yc"]
    if trace:
        kernel.last_results = res
    return out


# revision 16
# speedup vs baseline: 1.2599x; 1.2599x over previous
"""Multi-head attention (b=4, l=2048, d=1024, h=16) on 8 TRN2 NeuronCores.

Sharding: batch (4-way) x query-sequence (2-way) => 8 shards, no collectives.
Each core computes, for its (batch, query-half):
  - K/V projections for the full 2048-key sequence (duplicated across the
    2 cores sharing a batch), Q projection for its 1024 queries.
  - Scores in transposed orientation sT[k, q] = (k_h q_h^T) so the PV matmul
    contracts k on partitions; softmax without max-subtraction (scores ~N(0,1));
    the exp row-sum rides the PV matmul as a ones-column of v (M=65).
  - Output projection, writing its own [1024, 1024] slice of the output.

Compute dtype is fp16 (inputs cast on-device during DMA): matmuls stream at
1 cycle/row with fp32 PSUM accumulation and activations get hardware DMA
transpose (2-byte dtype). Softmax normalization stays in fp32. q/k/v live in
SBUF end to end; K-projection is emitted per head-pair interleaved with that
pair's attention so the PE fills the scalar-engine-bound exp phase.

Weights are passed to the device pre-transposed ([d_in, d_out]) - a host-side
layout choice.
"""

import sys
import types

import numpy as np

B, L, D, H, DK = 4, 2048, 1024, 16, 64
LQ = L // 2          # queries per core
P = 128              # partitions
DCH = D // P         # 8 d_in chunks
NPAIR = H // 2       # 8 head pairs
NKT = L // P         # 16 key tiles
N_CORES = 8
SCALE = 1.0 / np.sqrt(DK)

_NC_CACHE = {}
DEBUG_OUTPUTS = False


def _ensure_axon_hooks():
    """Register the NTFF profile hook module if the image's antenv lacks it."""
    try:
        import antenv  # noqa: F401
        from antenv import axon_hooks  # noqa: F401
        return
    except ImportError:
        pass
    try:
        import antenv

        mod = types.ModuleType("antenv.axon_hooks")
        mod._HOOK = None

        def set_axon_ntff_profile_hook(h):
            mod._HOOK = h

        def get_axon_ntff_profile_hook():
            return mod._HOOK

        mod.set_axon_ntff_profile_hook = set_axon_ntff_profile_hook
        mod.get_axon_ntff_profile_hook = get_axon_ntff_profile_hook
        sys.modules["antenv.axon_hooks"] = mod
        antenv.axon_hooks = mod
        from trn_agent_boot.trn_boot import _ntff_profile_via_ctypes

        set_axon_ntff_profile_hook(
            _ntff_profile_via_ctypes("/opt/axon/libaxon_pjrt.so")
        )
    except Exception:
        pass


def build_nc():
    import concourse.tile as tile
    from concourse import bacc, mybir
    from contextlib import ExitStack

    f32 = mybir.dt.float32
    f16 = mybir.dt.float16
    Exp = mybir.ActivationFunctionType.Exp

    nc = bacc.Bacc(
        "TRN2",
        target_bir_lowering=False,
        debug=False,
        enable_asserts=False,
        num_devices=N_CORES,
    )

    Qc = nc.dram_tensor("Qc", [LQ, D], f32, kind="ExternalInput").ap()
    Kc = nc.dram_tensor("Kc", [L, D], f32, kind="ExternalInput").ap()
    Vc = nc.dram_tensor("Vc", [L, D], f32, kind="ExternalInput").ap()
    WQT = nc.dram_tensor("WQT", [D, D], f32, kind="ExternalInput").ap()
    WKT = nc.dram_tensor("WKT", [D, D], f32, kind="ExternalInput").ap()
    WVT = nc.dram_tensor("WVT", [D, D], f32, kind="ExternalInput").ap()
    WOT = nc.dram_tensor("WOT", [D, D], f32, kind="ExternalInput").ap()
    Yc = nc.dram_tensor("Yc", [LQ, D], f32, kind="ExternalOutput").ap()
    dbg = {}
    if DEBUG_OUTPUTS:
        dbg["oT"] = nc.dram_tensor("dbg_oT", [D, LQ], f16, kind="ExternalOutput").ap()

    with tile.TileContext(nc) as tc, ExitStack() as top:
        dram = top.enter_context(tc.tile_pool(name="dram", bufs=1, space="DRAM"))
        Qh = dram.tile([LQ, D], f16)
        Kh = dram.tile([L, D], f16)
        Vh = dram.tile([L, D], f16)

        wpool = top.enter_context(tc.tile_pool(name="w", bufs=2))

        def load_weight(WT):
            w = wpool.tile([P, DCH * D], f16)
            nc.gpsimd.dma_start(
                w[:].rearrange("p (c o) -> p c o", c=DCH),
                WT.rearrange("(c p) o -> p c o", p=P),
            )
            return w

        # SWDGE casts, in consumption order (weights first for each phase)
        wv = load_weight(WVT)
        for half in range(2):
            r0, r1 = half * (L // 2), (half + 1) * (L // 2)
            nc.gpsimd.dma_start(Vh[r0:r1, :], Vc[r0:r1, :])
        wq = load_weight(WQT)
        nc.gpsimd.dma_start(Qh[:, :], Qc[:, :])
        wk = load_weight(WKT)
        for half in range(2):
            r0, r1 = half * (L // 2), (half + 1) * (L // 2)
            nc.gpsimd.dma_start(Kh[r0:r1, :], Kc[r0:r1, :])

        xtp = top.enter_context(tc.tile_pool(name="xt", bufs=2))
        stg = top.enter_context(tc.tile_pool(name="stg", bufs=4))
        psum = top.enter_context(tc.tile_pool(name="psum", bufs=1, space="PSUM"))
        vpool = top.enter_context(tc.tile_pool(name="vsb", bufs=1))
        qatt = top.enter_context(tc.tile_pool(name="qatt", bufs=NPAIR))
        katt = top.enter_context(tc.tile_pool(name="katt", bufs=2))
        epool = top.enter_context(tc.tile_pool(name="epool", bufs=3))
        npool = top.enter_context(tc.tile_pool(name="npool", bufs=4))
        rpool = top.enter_context(tc.tile_pool(name="rpool", bufs=4))
        otp = top.enter_context(tc.tile_pool(name="otp", bufs=NPAIR))
        ystg = top.enter_context(tc.tile_pool(name="ystg", bufs=3))

        def load_xt(Xh, l0, lblk, name):
            """xt[p, c*lblk + j] = Xh[l0+j, c*P+p] via hardware DMA transpose."""
            xt = xtp.tile([P, DCH * lblk], f16, tag="xt", name=name)
            xt3 = xt[:].rearrange("p (c l) -> p c l", c=DCH)
            for c in range(DCH):
                nc.sync.dma_start(
                    xt3[:, c, :],
                    Xh[l0 : l0 + lblk, P * c : P * (c + 1)],
                    transpose=True,
                )
            return xt

        # ---------------- V projection -> v_sb (SBUF-resident) -------------
        # v_sb[p, 1040*t + 130*pr + 65*h + s]:
        #   s<64 -> v[128*t + p, 128*pr + 64*h + s]; s==64 -> 1.0
        v_sb = vpool.tile([P, NKT * NPAIR * 130], f16)
        vones = v_sb[:].rearrange("p (q s) -> p q s", s=65)[:, :, 64:65]
        nc.gpsimd.memset(vones, 1.0)
        for half in range(2):
            l0 = half * (L // 2)
            xt = load_xt(Vh, l0, L // 2, f"xtv{half}")
            xt3 = xt[:].rearrange("p (c l) -> p c l", c=DCH)
            for i in range(8):
                t = 8 * half + i
                for nb in range(2):
                    pp = psum.tile([P, 512], f32, tag="pp", bufs=2,
                                   name=f"ppv{t}_{nb}")
                    for c in range(DCH):
                        nc.tensor.matmul(
                            pp[:],
                            xt3[:, c, P * i : P * (i + 1)],
                            wv[:, c * D + 512 * nb : c * D + 512 * (nb + 1)],
                            start=(c == 0),
                            stop=(c == DCH - 1),
                        )
                    # psum cols (4 pairs)(2 heads)(64) -> v_sb 65-strided
                    dst = v_sb[:].rearrange(
                        "p (t r s) -> p t r s", t=NKT, s=65
                    )[:, t, 8 * nb : 8 * (nb + 1), 0:64]
                    nc.vector.tensor_copy(
                        dst, pp[:].rearrange("p (r s) -> p r s", s=64)
                    )

        # ---------------- Q projection -> qt tiles (SBUF-resident) ---------
        xt = load_xt(Qh, 0, LQ, "xtq")
        xt3 = xt[:].rearrange("p (c l) -> p c l", c=DCH)
        qt_tiles = []
        for pr in range(NPAIR):
            qt = qatt.tile([P, LQ], f16, tag="qt", name=f"qt{pr}")
            qt_tiles.append(qt)
            for lb in range(2):
                pp = psum.tile([P, 512], f32, tag="pp", bufs=2,
                               name=f"ppq{pr}_{lb}")
                for c in range(DCH):
                    nc.tensor.matmul(
                        pp[:],
                        wq[:, c * D + P * pr : c * D + P * (pr + 1)],
                        xt3[:, c, 512 * lb : 512 * (lb + 1)],
                        start=(c == 0),
                        stop=(c == DCH - 1),
                    )
                nc.vector.tensor_copy(qt[:, 512 * lb : 512 * (lb + 1)], pp[:])

        # ------- K projection per pair, interleaved with attention ---------
        xtk = [load_xt(Kh, half * (L // 2), L // 2, f"xtk{half}")
               for half in range(2)]
        xtk3 = [x[:].rearrange("p (c l) -> p c l", c=DCH) for x in xtk]

        ot_tiles = []
        for pr in range(NPAIR):
            kt = katt.tile([P, L], f16, tag="kt", name=f"kt{pr}")
            for half in range(2):
                for lb in range(2):
                    pp = psum.tile([P, 512], f32, tag="pp", bufs=2,
                                   name=f"ppk{pr}_{half}_{lb}")
                    for c in range(DCH):
                        nc.tensor.matmul(
                            pp[:],
                            wk[:, c * D + P * pr : c * D + P * (pr + 1)],
                            xtk3[half][:, c, 512 * lb : 512 * (lb + 1)],
                            start=(c == 0),
                            stop=(c == DCH - 1),
                        )
                    nc.vector.tensor_copy(
                        kt[:, half * 1024 + 512 * lb : half * 1024 + 512 * (lb + 1)],
                        pp[:],
                    )

            qt = qt_tiles[pr]
            ot = otp.tile([P, LQ], f16, tag="ot", name=f"ot{pr}")
            ot_tiles.append(ot)
            for qs in range(2):
                qsl = qt[:, 512 * qs : 512 * (qs + 1)]
                pvA = psum.tile([P, 512], f32, tag="pv", bufs=2,
                                name=f"pvA{pr}_{qs}")
                pvB = psum.tile([P, 512], f32, tag="pv", bufs=2,
                                name=f"pvB{pr}_{qs}")
                for g in range(NKT):
                    sab = psum.tile([P, 1024], f32, tag="sab", bufs=2,
                                    name=f"sab{pr}_{qs}_{g}")
                    st, sp = (g == 0), (g == NKT - 1)
                    for h in range(2):
                        nc.tensor.matmul(
                            sab[:, 512 * h : 512 * (h + 1)],
                            kt[64 * h : 64 * (h + 1), P * g : P * (g + 1)],
                            qsl[64 * h : 64 * (h + 1), :],
                            start=True,
                            stop=True,
                        )
                    e = epool.tile([P, 1024], f16, tag="e", name=f"e{pr}_{qs}_{g}")
                    nc.scalar.activation(e[:], sab[:], Exp, scale=float(SCALE))
                    for h, pv in ((0, pvA), (1, pvB)):
                        nc.tensor.matmul(
                            pv[0:65, :],
                            v_sb[:, 1040 * g + 130 * pr + 65 * h :
                                 1040 * g + 130 * pr + 65 * (h + 1)],
                            e[:, 512 * h : 512 * (h + 1)],
                            start=st,
                            stop=sp,
                        )
                # normalize: rows 0:64 = sum(exp*v), row 64 = sum(exp)
                for h, pv in ((0, pvA), (1, pvB)):
                    tmp = npool.tile([P, 512], f32, tag="tmp",
                                     name=f"tmp{pr}_{qs}_{h}")
                    nc.vector.tensor_copy(tmp[0:65, :], pv[0:65, :])
                    # custom DVE ops + partition_broadcast need base 0
                    srow = rpool.tile([1, 512], f32, tag="srow",
                                      name=f"srow{pr}_{qs}_{h}")
                    nc.vector.tensor_copy(srow[0:1, :], tmp[64:65, :])
                    rec = rpool.tile([1, 512], f32, tag="rec",
                                     name=f"rec{pr}_{qs}_{h}")
                    nc.vector.reciprocal_approx_fast(rec[0:1, :], srow[0:1, :])
                    bc = rpool.tile([64, 512], f32, tag="bc",
                                    name=f"bc{pr}_{qs}_{h}")
                    nc.gpsimd.partition_broadcast(bc[:, :], rec[0:1, :])
                    nc.vector.tensor_mul(
                        ot[64 * h : 64 * (h + 1), 512 * qs : 512 * (qs + 1)],
                        tmp[0:64, :],
                        bc[:, :],
                    )

        if DEBUG_OUTPUTS:
            for pr in range(NPAIR):
                nc.gpsimd.dma_start(
                    dbg["oT"][P * pr : P * (pr + 1), :], ot_tiles[pr][:]
                )

        # ---------------- output projection ----------------
        wo = load_weight(WOT)
        for qt_i in range(LQ // P):
            for nb in range(2):
                pp = psum.tile([P, 512], f32, tag="pp", bufs=2,
                               name=f"ppo{qt_i}_{nb}")
                for pr in range(NPAIR):
                    nc.tensor.matmul(
                        pp[:],
                        ot_tiles[pr][:, P * qt_i : P * (qt_i + 1)],
                        wo[:, pr * D + 512 * nb : pr * D + 512 * (nb + 1)],
                        start=(pr == 0),
                        stop=(pr == NPAIR - 1),
                    )
                ys = ystg.tile([P, 512], f32, tag="ys", name=f"ys{qt_i}_{nb}")
                nc.vector.tensor_copy(ys[:], pp[:])
                nc.sync.dma_start(
                    Yc[P * qt_i : P * (qt_i + 1), 512 * nb : 512 * (nb + 1)],
                    ys[:],
                )

    nc.compile()
    return nc


def get_nc():
    if "nc" not in _NC_CACHE:
        _NC_CACHE["nc"] = build_nc()
    return _NC_CACHE["nc"]


def make_in_maps(Q, K, V, WQ, WK, WV, WO):
    Q = np.asarray(Q, dtype=np.float32)
    K = np.asarray(K, dtype=np.float32)
    V = np.asarray(V, dtype=np.float32)
    WQT = np.ascontiguousarray(np.asarray(WQ, dtype=np.float32).T)
    WKT = np.ascontiguousarray(np.asarray(WK, dtype=np.float32).T)
    WVT = np.ascontiguousarray(np.asarray(WV, dtype=np.float32).T)
    WOT = np.ascontiguousarray(np.asarray(WO, dtype=np.float32).T)
    in_maps = []
    for c in range(N_CORES):
        b, half = c // 2, c % 2
        in_maps.append(
            {
                "Qc": np.ascontiguousarray(Q[b, half * LQ : (half + 1) * LQ, :]),
                "Kc": np.ascontiguousarray(K[b]),
                "Vc": np.ascontiguousarray(V[b]),
                "WQT": WQT,
                "WKT": WKT,
                "WVT": WVT,
                "WOT": WOT,
            }
        )
    return in_maps


def kernel(Q, K, V, WQ, WK, WV, WO, trace=False):
    _ensure_axon_hooks()
    from concourse.bass_utils import run_bass_kernel_spmd

    nc = get_nc()
    in_maps = make_in_maps(Q, K, V, WQ, WK, WV, WO)
    res = run_bass_kernel_spmd(
        nc, in_maps, core_ids=list(range(N_CORES)), trace=trace
    )
    out = np.empty((B, L, D), dtype=np.float32)
    for c in range(N_CORES):
        b, half = c // 2, c % 2
        out[b, half * LQ : (half + 1) * LQ, :] = res.results[c]["Yc"]
    if trace:
        kernel.last_results = res
    return out


# revision 17
# speedup vs baseline: 1.5265x; 1.2116x over previous
"""Multi-head attention (b=4, l=2048, d=1024, h=16) on 8 TRN2 NeuronCores.

Sharding: batch (4-way) x query-sequence (2-way) => 8 shards, no collectives.
Each core computes, for its (batch, query-half):
  - K/V projections for the full 2048-key sequence (duplicated across the
    2 cores sharing a batch), Q projection for its 1024 queries.
  - Scores in transposed orientation sT[k, q] = (k_h q_h^T) so the PV matmul
    contracts k on partitions; softmax without max-subtraction (scores ~N(0,1));
    the exp row-sum rides the PV matmul as a ones-column of v (M=65).
  - Output projection, writing its own [1024, 1024] slice of the output.

Compute dtype is fp16 (inputs cast on-device during DMA): matmuls stream at
1 cycle/row with fp32 PSUM accumulation and activations get hardware DMA
transpose (2-byte dtype). Softmax normalization stays in fp32. q/k/v live in
SBUF end to end; K-projection is emitted per head-pair interleaved with that
pair's attention so the PE fills the scalar-engine-bound exp phase.

Weights are passed to the device pre-transposed ([d_in, d_out]) - a host-side
layout choice.
"""

import sys
import types

import numpy as np

B, L, D, H, DK = 4, 2048, 1024, 16, 64
LQ = L // 2          # queries per core
P = 128              # partitions
DCH = D // P         # 8 d_in chunks
NPAIR = H // 2       # 8 head pairs
NKT = L // P         # 16 key tiles
N_CORES = 8
SCALE = 1.0 / np.sqrt(DK)

_NC_CACHE = {}
DEBUG_OUTPUTS = False


def _ensure_axon_hooks():
    """Register the NTFF profile hook module if the image's antenv lacks it."""
    try:
        import antenv  # noqa: F401
        from antenv import axon_hooks  # noqa: F401
        return
    except ImportError:
        pass
    try:
        import antenv

        mod = types.ModuleType("antenv.axon_hooks")
        mod._HOOK = None

        def set_axon_ntff_profile_hook(h):
            mod._HOOK = h

        def get_axon_ntff_profile_hook():
            return mod._HOOK

        mod.set_axon_ntff_profile_hook = set_axon_ntff_profile_hook
        mod.get_axon_ntff_profile_hook = get_axon_ntff_profile_hook
        sys.modules["antenv.axon_hooks"] = mod
        antenv.axon_hooks = mod
        from trn_agent_boot.trn_boot import _ntff_profile_via_ctypes

        set_axon_ntff_profile_hook(
            _ntff_profile_via_ctypes("/opt/axon/libaxon_pjrt.so")
        )
    except Exception:
        pass


def build_nc():
    import concourse.tile as tile
    from concourse import bacc, mybir
    from contextlib import ExitStack

    f32 = mybir.dt.float32
    f16 = mybir.dt.float16
    Exp = mybir.ActivationFunctionType.Exp

    nc = bacc.Bacc(
        "TRN2",
        target_bir_lowering=False,
        debug=False,
        enable_asserts=False,
        num_devices=N_CORES,
    )

    Qc = nc.dram_tensor("Qc", [LQ, D], f32, kind="ExternalInput").ap()
    Kc = nc.dram_tensor("Kc", [L, D], f32, kind="ExternalInput").ap()
    Vc = nc.dram_tensor("Vc", [L, D], f32, kind="ExternalInput").ap()
    WQT = nc.dram_tensor("WQT", [D, D], f16, kind="ExternalInput").ap()
    WKT = nc.dram_tensor("WKT", [D, D], f16, kind="ExternalInput").ap()
    WVT = nc.dram_tensor("WVT", [D, D], f16, kind="ExternalInput").ap()
    WOT = nc.dram_tensor("WOT", [D, D], f16, kind="ExternalInput").ap()
    Yc = nc.dram_tensor("Yc", [LQ, D], f32, kind="ExternalOutput").ap()
    dbg = {}
    if DEBUG_OUTPUTS:
        dbg["oT"] = nc.dram_tensor("dbg_oT", [D, LQ], f16, kind="ExternalOutput").ap()

    with tile.TileContext(nc) as tc, ExitStack() as top:
        dram = top.enter_context(tc.tile_pool(name="dram", bufs=1, space="DRAM"))
        Qh = dram.tile([LQ, D], f16)
        Kh = dram.tile([L, D], f16)
        Vh = dram.tile([L, D], f16)

        wpool = top.enter_context(tc.tile_pool(name="w", bufs=2))

        def load_weight(WT):
            w = wpool.tile([P, DCH * D], f16)
            nc.sync.dma_start(
                w[:].rearrange("p (c o) -> p c o", c=DCH),
                WT.rearrange("(c p) o -> p c o", p=P),
            )
            return w

        # fp16 input staging: SWDGE casts in 512-row chunks, consumption order
        wv = load_weight(WVT)
        wq = load_weight(WQT)
        for r0 in range(0, L, 512):
            nc.gpsimd.dma_start(Vh[r0 : r0 + 512, :], Vc[r0 : r0 + 512, :])
        for r0 in range(0, LQ, 512):
            nc.gpsimd.dma_start(Qh[r0 : r0 + 512, :], Qc[r0 : r0 + 512, :])
        wk = load_weight(WKT)
        for r0 in range(0, L, 512):
            nc.gpsimd.dma_start(Kh[r0 : r0 + 512, :], Kc[r0 : r0 + 512, :])

        xtp = top.enter_context(tc.tile_pool(name="xt", bufs=2))
        stg = top.enter_context(tc.tile_pool(name="stg", bufs=4))
        psum = top.enter_context(tc.tile_pool(name="psum", bufs=1, space="PSUM"))
        vpool = top.enter_context(tc.tile_pool(name="vsb", bufs=1))
        qatt = top.enter_context(tc.tile_pool(name="qatt", bufs=NPAIR))
        katt = top.enter_context(tc.tile_pool(name="katt", bufs=2))
        epool = top.enter_context(tc.tile_pool(name="epool", bufs=4))
        npool = top.enter_context(tc.tile_pool(name="npool", bufs=4))
        rpool = top.enter_context(tc.tile_pool(name="rpool", bufs=4))
        otp = top.enter_context(tc.tile_pool(name="otp", bufs=NPAIR))
        ystg = top.enter_context(tc.tile_pool(name="ystg", bufs=3))

        def load_xt(Xh, l0, lblk, name):
            """xt[p, c*lblk + j] = Xh[l0+j, c*P+p] via hardware DMA transpose."""
            xt = xtp.tile([P, DCH * lblk], f16, tag="xt", name=name)
            xt3 = xt[:].rearrange("p (c l) -> p c l", c=DCH)
            for c in range(DCH):
                nc.sync.dma_start(
                    xt3[:, c, :],
                    Xh[l0 : l0 + lblk, P * c : P * (c + 1)],
                    transpose=True,
                )
            return xt

        # ---------------- V projection -> v_sb (SBUF-resident) -------------
        # v_sb[p, 1040*t + 130*pr + 65*h + s]:
        #   s<64 -> v[128*t + p, 128*pr + 64*h + s]; s==64 -> 1.0
        v_sb = vpool.tile([P, NKT * NPAIR * 130], f16)
        vones = v_sb[:].rearrange("p (q s) -> p q s", s=65)[:, :, 64:65]
        nc.gpsimd.memset(vones, 1.0)
        for half in range(2):
            l0 = half * (L // 2)
            xt = load_xt(Vh, l0, L // 2, f"xtv{half}")
            xt3 = xt[:].rearrange("p (c l) -> p c l", c=DCH)
            for i in range(8):
                t = 8 * half + i
                for nb in range(2):
                    pp = psum.tile([P, 512], f32, tag="pp", bufs=2,
                                   name=f"ppv{t}_{nb}")
                    for c in range(DCH):
                        nc.tensor.matmul(
                            pp[:],
                            xt3[:, c, P * i : P * (i + 1)],
                            wv[:, c * D + 512 * nb : c * D + 512 * (nb + 1)],
                            start=(c == 0),
                            stop=(c == DCH - 1),
                        )
                    # psum cols (4 pairs)(2 heads)(64) -> v_sb 65-strided
                    dst = v_sb[:].rearrange(
                        "p (t r s) -> p t r s", t=NKT, s=65
                    )[:, t, 8 * nb : 8 * (nb + 1), 0:64]
                    nc.vector.tensor_copy(
                        dst, pp[:].rearrange("p (r s) -> p r s", s=64)
                    )

        # ---------------- Q projection -> qt tiles (SBUF-resident) ---------
        xt = load_xt(Qh, 0, LQ, "xtq")
        xt3 = xt[:].rearrange("p (c l) -> p c l", c=DCH)
        qt_tiles = []
        for pr in range(NPAIR):
            qt = qatt.tile([P, LQ], f16, tag="qt", name=f"qt{pr}")
            qt_tiles.append(qt)
            for lb in range(2):
                pp = psum.tile([P, 512], f32, tag="pp", bufs=2,
                               name=f"ppq{pr}_{lb}")
                for c in range(DCH):
                    nc.tensor.matmul(
                        pp[:],
                        wq[:, c * D + P * pr : c * D + P * (pr + 1)],
                        xt3[:, c, 512 * lb : 512 * (lb + 1)],
                        start=(c == 0),
                        stop=(c == DCH - 1),
                    )
                nc.vector.tensor_copy(qt[:, 512 * lb : 512 * (lb + 1)], pp[:])

        # ------- K projection per pair, interleaved with attention ---------
        xtk = [load_xt(Kh, half * (L // 2), L // 2, f"xtk{half}")
               for half in range(2)]
        xtk3 = [x[:].rearrange("p (c l) -> p c l", c=DCH) for x in xtk]

        ot_tiles = []
        for pr in range(NPAIR):
            kt = katt.tile([P, L], f16, tag="kt", name=f"kt{pr}")
            for half in range(2):
                for lb in range(2):
                    pp = psum.tile([P, 512], f32, tag="pp", bufs=2,
                                   name=f"ppk{pr}_{half}_{lb}")
                    for c in range(DCH):
                        nc.tensor.matmul(
                            pp[:],
                            wk[:, c * D + P * pr : c * D + P * (pr + 1)],
                            xtk3[half][:, c, 512 * lb : 512 * (lb + 1)],
                            start=(c == 0),
                            stop=(c == DCH - 1),
                        )
                    nc.vector.tensor_copy(
                        kt[:, half * 1024 + 512 * lb : half * 1024 + 512 * (lb + 1)],
                        pp[:],
                    )

            qt = qt_tiles[pr]
            ot = otp.tile([P, LQ], f16, tag="ot", name=f"ot{pr}")
            ot_tiles.append(ot)
            for qs in range(2):
                qsl = qt[:, 512 * qs : 512 * (qs + 1)]
                pvA = psum.tile([P, 512], f32, tag="pv", bufs=2,
                                name=f"pvA{pr}_{qs}")
                pvB = psum.tile([P, 512], f32, tag="pv", bufs=2,
                                name=f"pvB{pr}_{qs}")
                for g in range(NKT):
                    sab = psum.tile([P, 1024], f32, tag="sab", bufs=2,
                                    name=f"sab{pr}_{qs}_{g}")
                    st, sp = (g == 0), (g == NKT - 1)
                    for h in range(2):
                        nc.tensor.matmul(
                            sab[:, 512 * h : 512 * (h + 1)],
                            kt[64 * h : 64 * (h + 1), P * g : P * (g + 1)],
                            qsl[64 * h : 64 * (h + 1), :],
                            start=True,
                            stop=True,
                        )
                    e = epool.tile([P, 1024], f16, tag="e", name=f"e{pr}_{qs}_{g}")
                    nc.scalar.activation(e[:], sab[:], Exp, scale=float(SCALE))
                    for h, pv in ((0, pvA), (1, pvB)):
                        nc.tensor.matmul(
                            pv[0:65, :],
                            v_sb[:, 1040 * g + 130 * pr + 65 * h :
                                 1040 * g + 130 * pr + 65 * (h + 1)],
                            e[:, 512 * h : 512 * (h + 1)],
                            start=st,
                            stop=sp,
                        )
                # normalize: rows 0:64 = sum(exp*v), row 64 = sum(exp)
                for h, pv in ((0, pvA), (1, pvB)):
                    tmp = npool.tile([P, 512], f32, tag="tmp",
                                     name=f"tmp{pr}_{qs}_{h}")
                    nc.vector.tensor_copy(tmp[0:65, :], pv[0:65, :])
                    # custom DVE ops + partition_broadcast need base 0
                    srow = rpool.tile([1, 512], f32, tag="srow",
                                      name=f"srow{pr}_{qs}_{h}")
                    nc.vector.tensor_copy(srow[0:1, :], tmp[64:65, :])
                    rec = rpool.tile([1, 512], f32, tag="rec",
                                     name=f"rec{pr}_{qs}_{h}")
                    nc.vector.reciprocal_approx_fast(rec[0:1, :], srow[0:1, :])
                    bc = rpool.tile([64, 512], f32, tag="bc",
                                    name=f"bc{pr}_{qs}_{h}")
                    nc.gpsimd.partition_broadcast(bc[:, :], rec[0:1, :])
                    nc.vector.tensor_mul(
                        ot[64 * h : 64 * (h + 1), 512 * qs : 512 * (qs + 1)],
                        tmp[0:64, :],
                        bc[:, :],
                    )

        if DEBUG_OUTPUTS:
            for pr in range(NPAIR):
                nc.gpsimd.dma_start(
                    dbg["oT"][P * pr : P * (pr + 1), :], ot_tiles[pr][:]
                )

        # ---------------- output projection ----------------
        wo = load_weight(WOT)
        for qt_i in range(LQ // P):
            for nb in range(2):
                pp = psum.tile([P, 512], f32, tag="pp", bufs=2,
                               name=f"ppo{qt_i}_{nb}")
                for pr in range(NPAIR):
                    nc.tensor.matmul(
                        pp[:],
                        ot_tiles[pr][:, P * qt_i : P * (qt_i + 1)],
                        wo[:, pr * D + 512 * nb : pr * D + 512 * (nb + 1)],
                        start=(pr == 0),
                        stop=(pr == NPAIR - 1),
                    )
                ys = ystg.tile([P, 512], f32, tag="ys", name=f"ys{qt_i}_{nb}")
                nc.vector.tensor_copy(ys[:], pp[:])
                nc.sync.dma_start(
                    Yc[P * qt_i : P * (qt_i + 1), 512 * nb : 512 * (nb + 1)],
                    ys[:],
                )

    nc.compile()
    return nc


def get_nc():
    if "nc" not in _NC_CACHE:
        _NC_CACHE["nc"] = build_nc()
    return _NC_CACHE["nc"]


def make_in_maps(Q, K, V, WQ, WK, WV, WO):
    Q = np.asarray(Q, dtype=np.float32)
    K = np.asarray(K, dtype=np.float32)
    V = np.asarray(V, dtype=np.float32)
    WQT = np.ascontiguousarray(np.asarray(WQ).T.astype(np.float16))
    WKT = np.ascontiguousarray(np.asarray(WK).T.astype(np.float16))
    WVT = np.ascontiguousarray(np.asarray(WV).T.astype(np.float16))
    WOT = np.ascontiguousarray(np.asarray(WO).T.astype(np.float16))
    in_maps = []
    for c in range(N_CORES):
        b, half = c // 2, c % 2
        in_maps.append(
            {
                "Qc": np.ascontiguousarray(Q[b, half * LQ : (half + 1) * LQ, :]),
                "Kc": np.ascontiguousarray(K[b]),
                "Vc": np.ascontiguousarray(V[b]),
                "WQT": WQT,
                "WKT": WKT,
                "WVT": WVT,
                "WOT": WOT,
            }
        )
    return in_maps


def kernel(Q, K, V, WQ, WK, WV, WO, trace=False):
    _ensure_axon_hooks()
    from concourse.bass_utils import run_bass_kernel_spmd

    nc = get_nc()
    in_maps = make_in_maps(Q, K, V, WQ, WK, WV, WO)
    res = run_bass_kernel_spmd(
        nc, in_maps, core_ids=list(range(N_CORES)), trace=trace
    )
    out = np.empty((B, L, D), dtype=np.float32)
    for c in range(N_CORES):
        b, half = c // 2, c % 2
        out[b, half * LQ : (half + 1) * LQ, :] = res.results[c]["Yc"]
    if trace:
        kernel.last_results = res
    return out
